# revision 10
# baseline (speedup 1.0000x reference)
"""Edge-MLP GNN message passing kernel for Trainium2 (8 NeuronCores).

Computes, for each edge e = (u, v):
    out[e] = sigmoid(relu(|x[u] - x[v]| @ W1 + b1) @ W2 + b2)

Strategy (data parallel over edges, x + weights replicated):
  - Per-edge SWDGE dma_gather is only used for the NON-shared endpoint
    (o-side): ~2.4ns/descriptor is a hard DMA-engine floor, so the
    shared endpoint (star center) never touches the DMA at all:
    its rows are produced by PE one-hot matmuls against an SBUF-resident
    (negated) copy of x, one matmul per block-run of star columns.
  - The host packs edges into "stars" (groups of r in {16,8,4,2} edges
    sharing an endpoint; |x_u - x_v| is symmetric so either endpoint
    can be the center) with a GLOBAL degree-aware greedy, then deals
    stars round-robin per (r, center-block) bucket across the 8 cores
    so all cores share one block-run structure (one SPMD NEFF).
  - sT = -x[center] columns are built in 512-group windows:
    DVE one-hot S = is_equal(iota, center_rel); PE matmul with the
    resident -x block; ACT copies PSUM -> fp16 SBUF window.
  - Per 2048-edge chunk: SWDGE gathers o-rows edge-major; PE transposes
    them into PSUM (feature-major); DVE de = oT + sT (0-stride read of
    the center column across the star's k slots); ACT dT = Abs(de);
    PE h = W1.T @ dT (two 512-col groups packed in PSUM halves);
    DVE relu(h+b1); PE 128-edge matmul with W2; ACT sigmoid -> out_sb.
  - idx tiles are DMA'd per chunk (no monolithic index load on the
    critical path); one DMA out at the end.
"""

import os
import sys

for _p in ("/opt/trn_rl_repo", "/root/.axon_site/_ro/trn_rl_repo"):
    if os.path.isdir(_p) and _p not in sys.path:
        sys.path.insert(0, _p)

import numpy as np

import concourse.bacc as bacc
import concourse.mybir as mybir
from concourse.bass import AP
from concourse.mybir import AluOpType
from concourse.tile import TileContext
from concourse.bass_utils import run_bass_kernel_spmd

N_NODES = 10000
N_EDGES = 640000
D_FEAT = 128
HID = 64
N_CORES = 8
N_BLOCKS = 80  # 10240 rows, last 240 are zero pad

CHUNK = 2048     # edges per v-gather chunk
WIN = 512        # star columns per sT window
LOOKAHEAD = 8    # edge-chunks of gather issued ahead of compute
IDX_AHEAD = 3    # chunks of idx-tile DMA issued ahead of the gather
N_QUEUES = 4
SCRATCH = 32768
REPS = (16, 8, 4, 2)

f16 = mybir.dt.float16
f32 = mybir.dt.float32
i16 = mybir.dt.int16

_NC_CACHE = {}


def _plan(region_groups, run_blocks):
    """Shared (SPMD-uniform) schedule.

    region_groups: tuple of (r, n_stars) incl. (1, n_singles);
        star counts are multiples of 128, identical across cores.
    run_blocks: per region, list of (col0, col1, block) star-col runs
        (identical across cores).
    Returns chunks, windows, win_of.
    """
    windows = []  # (u0, ncols, [(c0, c1, block) local runs])
    u_base = 0
    for (r, G), runs in zip(region_groups, run_blocks):
        w0 = 0
        while w0 < G:
            n = min(WIN, G - w0)
            loc = []
            for c0, c1, b in runs:
                a, z = max(c0, w0), min(c1, w0 + n)
                if a < z:
                    loc.append((a - w0, z - w0, b))
            windows.append((u_base + w0, n, loc))
            w0 += n
        u_base += G
    win_of = {}
    for wi, (u0, n, _) in enumerate(windows):
        for c in range(u0, u0 + n):
            win_of[c] = wi

    chunks = []  # (r, C, u0_global, v_off, col0)
    u_base = 0
    v_off = 0
    col0 = 0
    for r, G in region_groups:
        edges = r * G
        e = 0
        while e < edges:
            C = min(CHUNK, edges - e)
            chunks.append((r, C, u_base + e // r, v_off, col0))
            e += C
            v_off += C
            col0 += C // 128
        u_base += G
    return chunks, windows, win_of


def _build_nc(region_groups, run_blocks):
    T = sum(r * g for r, g in region_groups)
    U = sum(g for _, g in region_groups)
    n_out_cols = T // 128

    nc = bacc.Bacc(
        "TRN2",
        target_bir_lowering=False,
        num_swdge_queues=N_QUEUES,
        dynamic_dma_scratch_size=SCRATCH,
    )

    x16 = nc.dram_tensor("x16", [N_NODES, D_FEAT], f16, kind="ExternalInput")
    nxb_d = nc.dram_tensor("nxb", [128, N_BLOCKS * D_FEAT], f16,
                           kind="ExternalInput")
    idxv_d = nc.dram_tensor("idxv", [128, T // 16], i16, kind="ExternalInput")
    crow_d = nc.dram_tensor("crow", [128, U], f16, kind="ExternalInput")
    iota_d = nc.dram_tensor("iota", [128, 1], f16, kind="ExternalInput")
    w1_d = nc.dram_tensor("w1", [D_FEAT, HID], f16, kind="ExternalInput")
    w2_d = nc.dram_tensor("w2", [128, 1], f16, kind="ExternalInput")
    b1_d = nc.dram_tensor("b1", [128, 1], f32, kind="ExternalInput")
    b2_d = nc.dram_tensor("b2", [128, 1], f32, kind="ExternalInput")
    id_d = nc.dram_tensor("ident", [128, 128], f16, kind="ExternalInput")
    out_d = nc.dram_tensor("out", [128, n_out_cols], f32, kind="ExternalOutput")

    q_load = [0] * N_QUEUES
    chunks, windows, win_of = _plan(region_groups, run_blocks)

    with TileContext(nc) as tc:
        with (
            tc.tile_pool(name="const", bufs=1) as cpool,
            tc.tile_pool(name="gathv", bufs=10) as gvpool,
            tc.tile_pool(name="idxv", bufs=6) as ivpool,
            tc.tile_pool(name="stw", bufs=14) as stpool,
            tc.tile_pool(name="smat", bufs=2) as spool,
            tc.tile_pool(name="diff", bufs=3) as dpool,
            tc.tile_pool(name="dT", bufs=3) as dtpool,
            tc.tile_pool(name="hid", bufs=6) as hpool,
            tc.tile_pool(name="outp", bufs=1) as opool,
            tc.tile_pool(name="pst", bufs=2, space="PSUM") as tpool,
            tc.tile_pool(name="psq", bufs=2, space="PSUM") as qpool,
            tc.tile_pool(name="ps1", bufs=2, space="PSUM") as ppool,
            tc.tile_pool(name="ps2", bufs=2, space="PSUM") as p2pool,
        ):
            nxb = cpool.tile([128, N_BLOCKS * D_FEAT], f16, tag="nxb")
            crow = cpool.tile([128, U], f16, tag="crow")
            iota = cpool.tile([128, 1], f16, tag="iota")
            w1 = cpool.tile([D_FEAT, HID], f16, tag="w1")
            w2 = cpool.tile([128, 1], f16, tag="w2")
            b1 = cpool.tile([128, 1], f32, tag="b1")
            b2 = cpool.tile([128, 1], f32, tag="b2")
            ident = cpool.tile([128, 128], f16, tag="ident")
            out_sb = opool.tile([128, n_out_cols], f32, tag="osb")

            nc.sync.dma_start(iota[:], iota_d[:])
            nc.sync.dma_start(crow[:], crow_d[:])
            nc.sync.dma_start(nxb[:], nxb_d[:])
            nc.sync.dma_start(w1[:], w1_d[:])
            nc.sync.dma_start(w2[:], w2_d[:])
            nc.sync.dma_start(b1[:], b1_d[:])
            nc.sync.dma_start(b2[:], b2_d[:])
            nc.sync.dma_start(ident[:], id_d[:])

            def gather(dst_tile, n_idx, idx_tile):
                q = min(range(N_QUEUES), key=lambda i: q_load[i])
                q_load[q] += n_idx
                nc.gpsimd.dma_gather(
                    dst_tile[:, 0:n_idx].rearrange(
                        "p (a e) -> p a e", e=D_FEAT),
                    x16[:],
                    idx_tile[:, 0 : n_idx // 16],
                    n_idx,
                    n_idx,
                    elem_size=D_FEAT,
                    transpose=False,
                    single_packet=False,
                    queue_num=q,
                )

            # ---- sT window build: S one-hot -> PE matmul runs -> fp16 ----
            st_tiles = {}

            def build_window(wi):
                if wi in st_tiles or wi >= len(windows):
                    return
                u0, n, runs = windows[wi]
                S = spool.tile([128, WIN], f16, tag="S")
                io_ap = AP(iota.tensor, iota[:].offset,
                           [iota[:].ap[0], [0, n]])
                nc.vector.tensor_tensor(
                    S[:, 0:n], io_ap, crow[:, u0 : u0 + n],
                    AluOpType.is_equal,
                )
                pq = qpool.tile([128, WIN], f32, tag="pq")
                for c0, c1, b in runs:
                    nc.tensor.matmul(
                        pq[:, c0:c1],
                        nxb[:, b * D_FEAT : (b + 1) * D_FEAT],
                        S[:, c0:c1],
                        start=True, stop=True,
                    )
                st = stpool.tile([128, WIN], f16, tag="st")
                nc.scalar.activation(
                    st[:, 0:n], pq[:, 0:n],
                    mybir.ActivationFunctionType.Copy,
                )
                st_tiles[wi] = st

            chunk_gv = {}

            def compute(ch):
                """One edge chunk: transpose o, add sT, abs, MLP."""
                r, C, u0, v_off, col0 = ch
                gv = chunk_gv.pop(id(ch))
                dT = dtpool.tile([128, CHUNK], f16, tag="dT")
                for t in range(0, C, 1024):
                    nt = min(1024, C - t)
                    pt = tpool.tile([128, 1024], f16, tag="pt")
                    for j in range(nt // 128):
                        a = t + j * 128
                        nc.tensor.transpose(
                            pt[:, j * 128 : (j + 1) * 128],
                            gv[:, a : a + 128],
                            ident[:],
                        )
                    # de = oT + (-x[center]); center col broadcast over the
                    # star's k slots (0-stride) and 128 partitions-as-cols.
                    # Edge-col of this piece (region-relative): CO..CO+ncols;
                    # edge-col c belongs to star col c//r (k = c%r).
                    de = dpool.tile([128, 1024], f16, tag="de")
                    ncols_p = nt // 128
                    CO = t // 128            # edge-col within chunk
                    kcnt = min(r, ncols_p)   # k slots covered per star-col
                    jcnt_all = ncols_p // kcnt
                    j0 = 0
                    while j0 < jcnt_all:
                        # star base of the j0-th group-col in this piece;
                        # edge slot (group-col j, k, p) uses star j*128+p.
                        gc = u0 + ((CO + j0 * kcnt) // r) * 128
                        wi = win_of[gc]
                        w_u0, w_n, _ = windows[wi]
                        st = st_tiles[wi]
                        jn = min(jcnt_all - j0, (w_u0 + w_n - gc) // 128)
                        stv = st[:, gc - w_u0 : gc - w_u0 + jn * 128]
                        st4 = AP(
                            stv.tensor, stv.offset,
                            [stv.ap[0], [128, jn], [0, kcnt],
                             [1, 128]],
                        )
                        seg = j0 * kcnt * 128
                        n_here = jn * kcnt * 128
                        pt4 = pt[:, seg : seg + n_here].rearrange(
                            "p (j k f) -> p j k f", k=kcnt, f=128)
                        de4 = de[:, seg : seg + n_here].rearrange(
                            "p (j k f) -> p j k f", k=kcnt, f=128)
                        nc.vector.tensor_tensor(de4, st4, pt4,
                                                AluOpType.add)
                        j0 += jn
                    nc.scalar.activation(
                        dT[:, t : t + nt], de[:, 0:nt],
                        mybir.ActivationFunctionType.Abs,
                    )

                ncols = C // 128
                p2 = p2pool.tile([128, CHUNK // 128], f32, tag="p2")
                colc = 0
                for g in range(0, C, 1024):
                    nA = min(512, C - g)
                    nB = min(512, C - g - nA)
                    pm = ppool.tile([128, 512], f32, tag="pm")
                    nc.tensor.matmul(
                        pm[0:HID, 0:nA], w1[:], dT[:, g : g + nA],
                        start=True, stop=True,
                    )
                    if nB:
                        nc.tensor.matmul(
                            pm[HID:128, 0:nB], w1[:], dT[:, g + nA : g + nA + nB],
                            start=True, stop=True,
                        )
                    h = hpool.tile([128, 512], f16, tag="h")
                    if nB == nA:
                        nc.vector.tensor_scalar(
                            h[:, 0:nA], pm[:, 0:nA], b1[:], 0.0,
                            AluOpType.add, AluOpType.max,
                        )
                    else:
                        nc.vector.tensor_scalar(
                            h[0:HID, 0:nA], pm[0:HID, 0:nA], b1[0:HID, :], 0.0,
                            AluOpType.add, AluOpType.max,
                        )
                        if nB:
                            nc.vector.tensor_scalar(
                                h[HID:128, 0:nB], pm[HID:128, 0:nB],
                                b1[HID:128, :], 0.0,
                                AluOpType.add, AluOpType.max,
                            )
                    for j in range(nA // 128):
                        nc.tensor.matmul(
                            p2[:, colc : colc + 1],
                            h[0:HID, j * 128 : (j + 1) * 128],
                            w2[0:HID, :],
                            start=True, stop=True,
                        )
                        colc += 1
                    for j in range(nB // 128):
                        nc.tensor.matmul(
                            p2[:, colc : colc + 1],
                            h[HID:128, j * 128 : (j + 1) * 128],
                            w2[HID:128, :],
                            start=True, stop=True,
                        )
                        colc += 1
                nc.scalar.activation(
                    out_sb[:, col0 : col0 + ncols], p2[:, 0:ncols],
                    mybir.ActivationFunctionType.Sigmoid,
                    bias=b2[:], scale=1.0,
                )

            # ---- pipeline ----
            iv_tiles = {}

            def prefetch(i):
                if i >= len(chunks) or i in iv_tiles:
                    return
                r, C, u0, v_off, _ = chunks[i]
                it = ivpool.tile([128, CHUNK // 16], i16, tag="iv")
                nc.sync.dma_start(it[:, 0 : C // 16],
                                  idxv_d[:, v_off // 16 : (v_off + C) // 16])
                iv_tiles[i] = it

            def windows_of_chunk(ch):
                r, C, u0, _, _ = ch
                return list(range(win_of[u0], win_of[u0 + (C - 1) // r] + 1))

            pend = []
            for k in range(IDX_AHEAD):
                prefetch(k)
            for i, ch in enumerate(chunks):
                prefetch(i + IDX_AHEAD)
                for wi in windows_of_chunk(ch):
                    build_window(wi)
                gv = gvpool.tile([128, CHUNK], f16, tag="gv")
                gather(gv, ch[1], iv_tiles.pop(i))
                chunk_gv[id(ch)] = gv
                pend.append(ch)
                if len(pend) > LOOKAHEAD:
                    compute(pend.pop(0))
            while pend:
                compute(pend.pop(0))

            nc.sync.dma_start(out_d[:], out_sb[:])

    nc.finalize()
    return nc


def _get_nc(region_groups, run_blocks):
    key = (tuple(region_groups),
           tuple(tuple(rb) for rb in run_blocks))
    if key not in _NC_CACHE:
        _NC_CACHE[key] = _build_nc(region_groups, run_blocks)
    return _NC_CACHE[key]


def _interleave_idx(a):
    """[n] int array -> [128, n//16] int16 SWDGE index layout."""
    n = a.shape[0]
    m = a.reshape(n // 16, 16).T.astype(np.int16)  # [16, n/16]
    return np.tile(m, (8, 1))  # [128, n/16]


def _pack_global(u, v):
    """Global degree-aware greedy star cover over ALL edges.
    Returns dict rep -> list of (center, [edge ids]) and singles list
    (as (center, eid) pairs)."""
    E = len(u)
    incid = [[] for _ in range(N_NODES)]
    for e in range(E):
        incid[u[e]].append(e)
        if v[e] != u[e]:
            incid[v[e]].append(e)
    deg = [len(x) for x in incid]
    assigned = np.zeros(E, bool)
    maxd = max(deg)
    buckets = [[] for _ in range(maxd + 1)]
    for n in range(N_NODES):
        if deg[n]:
            buckets[deg[n]].append(n)
    resid = list(deg)
    groups = {r: [] for r in REPS}
    singles = []
    d = maxd
    while d > 0:
        bd = buckets[d]
        while bd:
            n = bd.pop()
            rn = resid[n]
            if rn != d:
                if rn > 0:
                    buckets[rn].append(n)
                continue
            avail = [e for e in incid[n] if not assigned[e]]
            take = 0
            for r in REPS:
                if len(avail) >= r:
                    take = r
                    break
            if take == 0:
                for e in avail:
                    assigned[e] = True
                    singles.append((n, e))
                    o = v[e] if u[e] == n else u[e]
                    if o != n:
                        resid[o] -= 1
                resid[n] = 0
                continue
            grp = avail[:take]
            groups[take].append((n, grp))
            for e in grp:
                assigned[e] = True
                o = v[e] if u[e] == n else u[e]
                if o != n:
                    resid[o] -= 1
            resid[n] -= take
            if resid[n] > 0:
                buckets[min(resid[n], d)].append(n)
        d -= 1
    return groups, singles


def prep_in_maps(x, indices, W1, b1, W2, b2):
    x32 = np.ascontiguousarray(np.asarray(x, dtype=np.float32))
    x16 = x32.astype(np.float16)
    idx = np.asarray(indices)
    su, sv = idx[0], idx[1]
    w1 = np.asarray(W1, dtype=np.float32).astype(np.float16)
    w2c = np.asarray(W2, dtype=np.float32).astype(np.float16).reshape(HID, 1)
    w2s = np.concatenate([w2c, w2c], axis=0)
    b1c = np.asarray(b1, dtype=np.float32).reshape(HID, 1)
    b1s = np.concatenate([b1c, b1c], axis=0)
    b2s = np.full((128, 1), np.asarray(b2, dtype=np.float32).reshape(-1)[0],
                  dtype=np.float32)
    ident = np.eye(128, dtype=np.float16)
    iota_c = np.arange(128, dtype=np.float16).reshape(128, 1)

    # resident negated x, block-column layout:
    # nxb[n, 128*b + f] = -x[128b + n, f]
    xpad = np.zeros((N_BLOCKS * 128, D_FEAT), np.float32)
    xpad[:N_NODES] = x32
    nxb = (-xpad).reshape(N_BLOCKS, 128, D_FEAT).transpose(1, 0, 2) \
        .reshape(128, N_BLOCKS * D_FEAT).astype(np.float16)

    groups, singles = _pack_global(su, sv)
    all_regions = list(REPS) + [1]
    stars = {r: groups[r] for r in REPS}
    stars[1] = [(n, [e]) for n, e in singles]

    # deal stars per (r, block) across cores; identical counts via pad
    per_core = {r: [[] for _ in range(N_CORES)] for r in all_regions}
    nrb = {}
    for r in all_regions:
        byb = [[] for _ in range(N_BLOCKS)]
        for n, grp in stars[r]:
            byb[n // 128].append((n, grp))
        for b in range(N_BLOCKS):
            lst = byb[b]
            tgt = (len(lst) + N_CORES - 1) // N_CORES
            nrb[(r, b)] = tgt
            for c in range(N_CORES):
                part = lst[c * tgt : (c + 1) * tgt]
                part = part + [(b * 128, None)] * (tgt - len(part))
                per_core[r][c].extend(part)

    region_groups = []
    run_blocks = []
    for r in all_regions:
        G = sum(nrb[(r, b)] for b in range(N_BLOCKS))
        Gp = (G + 127) // 128 * 128
        extra = Gp - G
        nrb[(r, N_BLOCKS - 1)] += extra
        for c in range(N_CORES):
            per_core[r][c].extend([((N_BLOCKS - 1) * 128, None)] * extra)
        region_groups.append((r, Gp))
        runs = []
        c0 = 0
        for b in range(N_BLOCKS):
            n = nrb[(r, b)]
            if n:
                runs.append((c0, c0 + n, b))
                c0 += n
        run_blocks.append(tuple(runs))
    region_groups = tuple(region_groups)

    U = sum(g for _, g in region_groups)
    T = sum(r * g for r, g in region_groups)

    in_maps = []
    perms = []
    for c in range(N_CORES):
        crow = np.zeros(U, np.float16)
        vvals = np.zeros(T, np.int64)
        perm = np.full(T, -1, np.int64)
        u_off = 0
        e_off = 0
        for (r, Gf) in region_groups:
            lst = per_core[r][c]
            assert len(lst) == Gf, (r, len(lst), Gf)
            for t, (n, grp) in enumerate(lst):
                p, j = t % 128, t // 128
                crow[u_off + t] = np.float16(n % 128)
                if grp is None:
                    continue
                for k, e in enumerate(grp):
                    pos = e_off + (r * j + k) * 128 + p
                    perm[pos] = e
                    vvals[pos] = sv[e] if su[e] == n else su[e]
            u_off += Gf
            e_off += r * Gf

        in_maps.append({
            "x16": x16,
            "nxb": nxb,
            "idxv": _interleave_idx(vvals),
            "crow": np.tile(crow.reshape(1, U), (128, 1)),
            "iota": iota_c,
            "w1": w1,
            "w2": w2s,
            "b1": b1s,
            "b2": b2s,
            "ident": ident,
        })
        perms.append(perm)
    return region_groups, run_blocks, in_maps, perms


def run_hw(x, indices, W1, b1, W2, b2, trace=False, **kw):
    region_groups, run_blocks, in_maps, perms = prep_in_maps(
        x, indices, W1, b1, W2, b2)
    nc = _get_nc(region_groups, run_blocks)
    res = run_bass_kernel_spmd(
        nc, in_maps, core_ids=list(range(N_CORES)), trace=trace, **kw
    )
    out = np.empty(N_EDGES, np.float32)
    for c in range(N_CORES):
        o = np.asarray(res.results[c]["out"])  # [128, T/128]
        slots = o.T.reshape(-1)  # slot s = col*128 + p
        perm = perms[c]
        mask = perm >= 0
        out[perm[mask]] = slots[mask]
    return out, res


def kernel(x, indices, W1, b1, W2, b2):
    out, _ = run_hw(x, indices, W1, b1, W2, b2, trace=False)
    return out.astype(np.float32)


# revision 11
# speedup vs baseline: 1.1156x; 1.1156x over previous
"""Edge-MLP GNN message passing kernel for Trainium2 (8 NeuronCores).

Computes, for each edge e = (u, v):
    out[e] = sigmoid(relu(|x[u] - x[v]| @ W1 + b1) @ W2 + b2)

Strategy (data parallel over edges, x + weights replicated):
  - Per-edge SWDGE dma_gather is only used for the NON-shared endpoint
    (o-side): ~2.4ns/descriptor is a hard DMA-engine floor, so the
    shared endpoint (star center) never touches the DMA at all:
    its rows are produced by PE one-hot matmuls against an SBUF-resident
    (negated) copy of x, one matmul per block-run of star columns.
  - The host packs edges into "stars" (groups of r in {16,8,4,2} edges
    sharing an endpoint; |x_u - x_v| is symmetric so either endpoint
    can be the center) with a GLOBAL degree-aware greedy, then deals
    stars round-robin per (r, center-block) bucket across the 8 cores
    so all cores share one block-run structure (one SPMD NEFF).
  - sT = -x[center] columns are built in 512-group windows:
    DVE one-hot S = is_equal(iota, center_rel); PE matmul with the
    resident -x block; ACT copies PSUM -> fp16 SBUF window.
  - Per 2048-edge chunk: SWDGE gathers o-rows edge-major; PE transposes
    them into PSUM (feature-major); DVE de = oT + sT (0-stride read of
    the center column across the star's k slots); ACT dT = Abs(de);
    PE h = W1.T @ dT (two 512-col groups packed in PSUM halves);
    DVE relu(h+b1); PE 128-edge matmul with W2; ACT sigmoid -> out_sb.
  - idx tiles are DMA'd per chunk (no monolithic index load on the
    critical path); one DMA out at the end.
"""

import os
import sys

for _p in ("/opt/trn_rl_repo", "/root/.axon_site/_ro/trn_rl_repo"):
    if os.path.isdir(_p) and _p not in sys.path:
        sys.path.insert(0, _p)

import numpy as np

import concourse.bacc as bacc
import concourse.mybir as mybir
from concourse.bass import AP
from concourse.mybir import AluOpType
from concourse.tile import TileContext
from concourse.bass_utils import run_bass_kernel_spmd

N_NODES = 10000
N_EDGES = 640000
D_FEAT = 128
HID = 64
N_CORES = 8
N_BLOCKS = 80  # 10240 rows, last 240 are zero pad

CHUNK = 2048     # edges per v-gather chunk
WIN = 512        # star columns per sT window
LOOKAHEAD = 2    # edge-chunks of gather issued ahead of compute
IDX_AHEAD = 3    # chunks of idx-tile DMA issued ahead of the gather
N_QUEUES = 4
SCRATCH = 32768
REPS = (16, 8, 4, 2)

f16 = mybir.dt.float16
f32 = mybir.dt.float32
i16 = mybir.dt.int16

_NC_CACHE = {}


def _plan(region_groups, run_blocks):
    """Shared (SPMD-uniform) schedule.

    region_groups: tuple of (r, n_stars) incl. (1, n_singles);
        star counts are multiples of 128, identical across cores.
    run_blocks: per region, list of (col0, col1, block) star-col runs
        (identical across cores).
    Returns chunks, windows, win_of.
    """
    windows = []  # (u0, ncols, [(c0, c1, block) local runs])
    u_base = 0
    for (r, G), runs in zip(region_groups, run_blocks):
        w0 = 0
        while w0 < G:
            n = min(WIN, G - w0)
            loc = []
            for c0, c1, b in runs:
                a, z = max(c0, w0), min(c1, w0 + n)
                if a < z:
                    loc.append((a - w0, z - w0, b))
            windows.append((u_base + w0, n, loc))
            w0 += n
        u_base += G
    win_of = {}
    for wi, (u0, n, _) in enumerate(windows):
        for c in range(u0, u0 + n):
            win_of[c] = wi

    chunks = []  # (r, C, u0_global, v_off, col0)
    u_base = 0
    v_off = 0
    col0 = 0
    for r, G in region_groups:
        edges = r * G
        e = 0
        while e < edges:
            C = min(CHUNK, edges - e)
            chunks.append((r, C, u_base + e // r, v_off, col0))
            e += C
            v_off += C
            col0 += C // 128
        u_base += G
    return chunks, windows, win_of


def _build_nc(region_groups, run_blocks):
    T = sum(r * g for r, g in region_groups)
    U = sum(g for _, g in region_groups)
    n_out_cols = T // 128

    nc = bacc.Bacc(
        "TRN2",
        target_bir_lowering=False,
        num_swdge_queues=N_QUEUES,
        dynamic_dma_scratch_size=SCRATCH,
    )

    x16 = nc.dram_tensor("x16", [N_NODES, D_FEAT], f16, kind="ExternalInput")
    nxb_d = nc.dram_tensor("nxb", [128, N_BLOCKS * D_FEAT], f16,
                           kind="ExternalInput")
    idxv_d = nc.dram_tensor("idxv", [128, T // 16], i16, kind="ExternalInput")
    crow_d = nc.dram_tensor("crow", [128, U], f16, kind="ExternalInput")
    iota_d = nc.dram_tensor("iota", [128, 1], f16, kind="ExternalInput")
    w1_d = nc.dram_tensor("w1", [D_FEAT, HID], f16, kind="ExternalInput")
    w2_d = nc.dram_tensor("w2", [128, 1], f16, kind="ExternalInput")
    b1_d = nc.dram_tensor("b1", [128, 1], f32, kind="ExternalInput")
    b2_d = nc.dram_tensor("b2", [128, 1], f32, kind="ExternalInput")
    id_d = nc.dram_tensor("ident", [128, 128], f16, kind="ExternalInput")
    out_d = nc.dram_tensor("out", [128, n_out_cols], f32, kind="ExternalOutput")

    q_load = [0] * N_QUEUES
    chunks, windows, win_of = _plan(region_groups, run_blocks)

    with TileContext(nc) as tc:
        with (
            tc.tile_pool(name="const", bufs=1) as cpool,
            tc.tile_pool(name="gathv", bufs=10) as gvpool,
            tc.tile_pool(name="idxv", bufs=6) as ivpool,
            tc.tile_pool(name="stw", bufs=14) as stpool,
            tc.tile_pool(name="smat", bufs=2) as spool,
            tc.tile_pool(name="diff", bufs=3) as dpool,
            tc.tile_pool(name="dT", bufs=3) as dtpool,
            tc.tile_pool(name="hid", bufs=6) as hpool,
            tc.tile_pool(name="outp", bufs=1) as opool,
            tc.tile_pool(name="pst", bufs=2, space="PSUM") as tpool,
            tc.tile_pool(name="psq", bufs=2, space="PSUM") as qpool,
            tc.tile_pool(name="ps1", bufs=2, space="PSUM") as ppool,
            tc.tile_pool(name="ps2", bufs=2, space="PSUM") as p2pool,
        ):
            nxb = cpool.tile([128, N_BLOCKS * D_FEAT], f16, tag="nxb")
            crow = cpool.tile([128, U], f16, tag="crow")
            iota = cpool.tile([128, 1], f16, tag="iota")
            w1 = cpool.tile([D_FEAT, HID], f16, tag="w1")
            w2 = cpool.tile([128, 1], f16, tag="w2")
            b1 = cpool.tile([128, 1], f32, tag="b1")
            b2 = cpool.tile([128, 1], f32, tag="b2")
            ident = cpool.tile([128, 128], f16, tag="ident")
            out_sb = opool.tile([128, n_out_cols], f32, tag="osb")

            def load_consts():
                nc.sync.dma_start(iota[:], iota_d[:])
                nc.sync.dma_start(crow[:], crow_d[:])
                nc.sync.dma_start(nxb[:], nxb_d[:])
                nc.sync.dma_start(w1[:], w1_d[:])
                nc.sync.dma_start(w2[:], w2_d[:])
                nc.sync.dma_start(b1[:], b1_d[:])
                nc.sync.dma_start(b2[:], b2_d[:])
                nc.sync.dma_start(ident[:], id_d[:])

            def gather(dst_tile, n_idx, idx_tile):
                q = min(range(N_QUEUES), key=lambda i: q_load[i])
                q_load[q] += n_idx
                nc.gpsimd.dma_gather(
                    dst_tile[:, 0:n_idx].rearrange(
                        "p (a e) -> p a e", e=D_FEAT),
                    x16[:],
                    idx_tile[:, 0 : n_idx // 16],
                    n_idx,
                    n_idx,
                    elem_size=D_FEAT,
                    transpose=False,
                    single_packet=False,
                    queue_num=q,
                )

            # ---- sT window build: S one-hot -> PE matmul runs -> fp16 ----
            st_tiles = {}

            def build_window(wi):
                if wi in st_tiles or wi >= len(windows):
                    return
                u0, n, runs = windows[wi]
                S = spool.tile([128, WIN], f16, tag="S")
                io_ap = AP(iota.tensor, iota[:].offset,
                           [iota[:].ap[0], [0, n]])
                nc.vector.tensor_tensor(
                    S[:, 0:n], io_ap, crow[:, u0 : u0 + n],
                    AluOpType.is_equal,
                )
                pq = qpool.tile([128, WIN], f32, tag="pq")
                for c0, c1, b in runs:
                    nc.tensor.matmul(
                        pq[:, c0:c1],
                        nxb[:, b * D_FEAT : (b + 1) * D_FEAT],
                        S[:, c0:c1],
                        start=True, stop=True,
                    )
                st = stpool.tile([128, WIN], f16, tag="st")
                nc.scalar.activation(
                    st[:, 0:n], pq[:, 0:n],
                    mybir.ActivationFunctionType.Copy,
                )
                st_tiles[wi] = st

            chunk_gv = {}

            def compute(ch):
                """One edge chunk: transpose o, add sT, abs, MLP."""
                r, C, u0, v_off, col0 = ch
                gv = chunk_gv.pop(id(ch))
                dT = dtpool.tile([128, CHUNK], f16, tag="dT")
                for t in range(0, C, 1024):
                    nt = min(1024, C - t)
                    pt = tpool.tile([128, 1024], f16, tag="pt")
                    for j in range(nt // 128):
                        a = t + j * 128
                        nc.tensor.transpose(
                            pt[:, j * 128 : (j + 1) * 128],
                            gv[:, a : a + 128],
                            ident[:],
                        )
                    # de = oT + (-x[center]); center col broadcast over the
                    # star's k slots (0-stride) and 128 partitions-as-cols.
                    # Edge-col of this piece (region-relative): CO..CO+ncols;
                    # edge-col c belongs to star col c//r (k = c%r).
                    de = dpool.tile([128, 1024], f16, tag="de")
                    ncols_p = nt // 128
                    CO = t // 128            # edge-col within chunk
                    kcnt = min(r, ncols_p)   # k slots covered per star-col
                    jcnt_all = ncols_p // kcnt
                    j0 = 0
                    while j0 < jcnt_all:
                        # star base of the j0-th group-col in this piece;
                        # edge slot (group-col j, k, p) uses star j*128+p.
                        gc = u0 + ((CO + j0 * kcnt) // r) * 128
                        wi = win_of[gc]
                        w_u0, w_n, _ = windows[wi]
                        st = st_tiles[wi]
                        jn = min(jcnt_all - j0, (w_u0 + w_n - gc) // 128)
                        stv = st[:, gc - w_u0 : gc - w_u0 + jn * 128]
                        st4 = AP(
                            stv.tensor, stv.offset,
                            [stv.ap[0], [128, jn], [0, kcnt],
                             [1, 128]],
                        )
                        seg = j0 * kcnt * 128
                        n_here = jn * kcnt * 128
                        pt4 = pt[:, seg : seg + n_here].rearrange(
                            "p (j k f) -> p j k f", k=kcnt, f=128)
                        de4 = de[:, seg : seg + n_here].rearrange(
                            "p (j k f) -> p j k f", k=kcnt, f=128)
                        nc.vector.tensor_tensor(de4, st4, pt4,
                                                AluOpType.add)
                        j0 += jn
                    nc.scalar.activation(
                        dT[:, t : t + nt], de[:, 0:nt],
                        mybir.ActivationFunctionType.Abs,
                    )

                ncols = C // 128
                p2 = p2pool.tile([128, CHUNK // 128], f32, tag="p2")
                colc = 0
                for g in range(0, C, 1024):
                    nA = min(512, C - g)
                    nB = min(512, C - g - nA)
                    pm = ppool.tile([128, 512], f32, tag="pm")
                    nc.tensor.matmul(
                        pm[0:HID, 0:nA], w1[:], dT[:, g : g + nA],
                        start=True, stop=True,
                    )
                    if nB:
                        nc.tensor.matmul(
                            pm[HID:128, 0:nB], w1[:], dT[:, g + nA : g + nA + nB],
                            start=True, stop=True,
                        )
                    h = hpool.tile([128, 512], f16, tag="h")
                    if nB == nA:
                        nc.vector.tensor_scalar(
                            h[:, 0:nA], pm[:, 0:nA], b1[:], 0.0,
                            AluOpType.add, AluOpType.max,
                        )
                    else:
                        nc.vector.tensor_scalar(
                            h[0:HID, 0:nA], pm[0:HID, 0:nA], b1[0:HID, :], 0.0,
                            AluOpType.add, AluOpType.max,
                        )
                        if nB:
                            nc.vector.tensor_scalar(
                                h[HID:128, 0:nB], pm[HID:128, 0:nB],
                                b1[HID:128, :], 0.0,
                                AluOpType.add, AluOpType.max,
                            )
                    for j in range(nA // 128):
                        nc.tensor.matmul(
                            p2[:, colc : colc + 1],
                            h[0:HID, j * 128 : (j + 1) * 128],
                            w2[0:HID, :],
                            start=True, stop=True,
                        )
                        colc += 1
                    for j in range(nB // 128):
                        nc.tensor.matmul(
                            p2[:, colc : colc + 1],
                            h[HID:128, j * 128 : (j + 1) * 128],
                            w2[HID:128, :],
                            start=True, stop=True,
                        )
                        colc += 1
                nc.scalar.activation(
                    out_sb[:, col0 : col0 + ncols], p2[:, 0:ncols],
                    mybir.ActivationFunctionType.Sigmoid,
                    bias=b2[:], scale=1.0,
                )

            # ---- pipeline ----
            iv_tiles = {}

            def prefetch(i):
                if i >= len(chunks) or i in iv_tiles:
                    return
                r, C, u0, v_off, _ = chunks[i]
                it = ivpool.tile([128, CHUNK // 16], i16, tag="iv")
                nc.sync.dma_start(it[:, 0 : C // 16],
                                  idxv_d[:, v_off // 16 : (v_off + C) // 16])
                iv_tiles[i] = it

            def windows_of_chunk(ch):
                r, C, u0, _, _ = ch
                return list(range(win_of[u0], win_of[u0 + (C - 1) // r] + 1))

            pend = []
            for k in range(IDX_AHEAD):
                prefetch(k)
            load_consts()
            for i, ch in enumerate(chunks):
                prefetch(i + IDX_AHEAD)
                for wi in windows_of_chunk(ch):
                    build_window(wi)
                gv = gvpool.tile([128, CHUNK], f16, tag="gv")
                gather(gv, ch[1], iv_tiles.pop(i))
                chunk_gv[id(ch)] = gv
                pend.append(ch)
                if len(pend) > LOOKAHEAD:
                    compute(pend.pop(0))
            while pend:
                compute(pend.pop(0))

            nc.sync.dma_start(out_d[:], out_sb[:])

    nc.finalize()
    return nc


def _get_nc(region_groups, run_blocks):
    key = (tuple(region_groups),
           tuple(tuple(rb) for rb in run_blocks))
    if key not in _NC_CACHE:
        _NC_CACHE[key] = _build_nc(region_groups, run_blocks)
    return _NC_CACHE[key]


def _interleave_idx(a):
    """[n] int array -> [128, n//16] int16 SWDGE index layout."""
    n = a.shape[0]
    m = a.reshape(n // 16, 16).T.astype(np.int16)  # [16, n/16]
    return np.tile(m, (8, 1))  # [128, n/16]


def _pack_global(u, v):
    """Global degree-aware greedy star cover over ALL edges.
    Returns dict rep -> list of (center, [edge ids]) and singles list
    (as (center, eid) pairs)."""
    E = len(u)
    incid = [[] for _ in range(N_NODES)]
    for e in range(E):
        incid[u[e]].append(e)
        if v[e] != u[e]:
            incid[v[e]].append(e)
    deg = [len(x) for x in incid]
    assigned = np.zeros(E, bool)
    maxd = max(deg)
    buckets = [[] for _ in range(maxd + 1)]
    for n in range(N_NODES):
        if deg[n]:
            buckets[deg[n]].append(n)
    resid = list(deg)
    groups = {r: [] for r in REPS}
    singles = []
    d = maxd
    while d > 0:
        bd = buckets[d]
        while bd:
            n = bd.pop()
            rn = resid[n]
            if rn != d:
                if rn > 0:
                    buckets[rn].append(n)
                continue
            avail = [e for e in incid[n] if not assigned[e]]
            take = 0
            for r in REPS:
                if len(avail) >= r:
                    take = r
                    break
            if take == 0:
                for e in avail:
                    assigned[e] = True
                    singles.append((n, e))
                    o = v[e] if u[e] == n else u[e]
                    if o != n:
                        resid[o] -= 1
                resid[n] = 0
                continue
            grp = avail[:take]
            groups[take].append((n, grp))
            for e in grp:
                assigned[e] = True
                o = v[e] if u[e] == n else u[e]
                if o != n:
                    resid[o] -= 1
            resid[n] -= take
            if resid[n] > 0:
                buckets[min(resid[n], d)].append(n)
        d -= 1
    return groups, singles


def prep_in_maps(x, indices, W1, b1, W2, b2):
    x32 = np.ascontiguousarray(np.asarray(x, dtype=np.float32))
    x16 = x32.astype(np.float16)
    idx = np.asarray(indices)
    su, sv = idx[0], idx[1]
    w1 = np.asarray(W1, dtype=np.float32).astype(np.float16)
    w2c = np.asarray(W2, dtype=np.float32).astype(np.float16).reshape(HID, 1)
    w2s = np.concatenate([w2c, w2c], axis=0)
    b1c = np.asarray(b1, dtype=np.float32).reshape(HID, 1)
    b1s = np.concatenate([b1c, b1c], axis=0)
    b2s = np.full((128, 1), np.asarray(b2, dtype=np.float32).reshape(-1)[0],
                  dtype=np.float32)
    ident = np.eye(128, dtype=np.float16)
    iota_c = np.arange(128, dtype=np.float16).reshape(128, 1)

    # resident negated x, block-column layout:
    # nxb[n, 128*b + f] = -x[128b + n, f]
    xpad = np.zeros((N_BLOCKS * 128, D_FEAT), np.float32)
    xpad[:N_NODES] = x32
    nxb = (-xpad).reshape(N_BLOCKS, 128, D_FEAT).transpose(1, 0, 2) \
        .reshape(128, N_BLOCKS * D_FEAT).astype(np.float16)

    groups, singles = _pack_global(su, sv)
    all_regions = list(REPS) + [1]
    stars = {r: groups[r] for r in REPS}
    stars[1] = [(n, [e]) for n, e in singles]

    # deal stars per (r, block) across cores; identical counts via pad
    per_core = {r: [[] for _ in range(N_CORES)] for r in all_regions}
    nrb = {}
    for r in all_regions:
        byb = [[] for _ in range(N_BLOCKS)]
        for n, grp in stars[r]:
            byb[n // 128].append((n, grp))
        for b in range(N_BLOCKS):
            lst = byb[b]
            tgt = (len(lst) + N_CORES - 1) // N_CORES
            nrb[(r, b)] = tgt
            for c in range(N_CORES):
                part = lst[c * tgt : (c + 1) * tgt]
                part = part + [(b * 128, None)] * (tgt - len(part))
                per_core[r][c].extend(part)

    region_groups = []
    run_blocks = []
    for r in all_regions:
        G = sum(nrb[(r, b)] for b in range(N_BLOCKS))
        Gp = (G + 127) // 128 * 128
        extra = Gp - G
        nrb[(r, N_BLOCKS - 1)] += extra
        for c in range(N_CORES):
            per_core[r][c].extend([((N_BLOCKS - 1) * 128, None)] * extra)
        region_groups.append((r, Gp))
        runs = []
        c0 = 0
        for b in range(N_BLOCKS):
            n = nrb[(r, b)]
            if n:
                runs.append((c0, c0 + n, b))
                c0 += n
        run_blocks.append(tuple(runs))
    region_groups = tuple(region_groups)

    U = sum(g for _, g in region_groups)
    T = sum(r * g for r, g in region_groups)

    in_maps = []
    perms = []
    for c in range(N_CORES):
        crow = np.zeros(U, np.float16)
        vvals = np.zeros(T, np.int64)
        perm = np.full(T, -1, np.int64)
        u_off = 0
        e_off = 0
        for (r, Gf) in region_groups:
            lst = per_core[r][c]
            assert len(lst) == Gf, (r, len(lst), Gf)
            for t, (n, grp) in enumerate(lst):
                p, j = t % 128, t // 128
                crow[u_off + t] = np.float16(n % 128)
                if grp is None:
                    continue
                for k, e in enumerate(grp):
                    pos = e_off + (r * j + k) * 128 + p
                    perm[pos] = e
                    vvals[pos] = sv[e] if su[e] == n else su[e]
            u_off += Gf
            e_off += r * Gf

        in_maps.append({
            "x16": x16,
            "nxb": nxb,
            "idxv": _interleave_idx(vvals),
            "crow": np.tile(crow.reshape(1, U), (128, 1)),
            "iota": iota_c,
            "w1": w1,
            "w2": w2s,
            "b1": b1s,
            "b2": b2s,
            "ident": ident,
        })
        perms.append(perm)
    return region_groups, run_blocks, in_maps, perms


def run_hw(x, indices, W1, b1, W2, b2, trace=False, **kw):
    region_groups, run_blocks, in_maps, perms = prep_in_maps(
        x, indices, W1, b1, W2, b2)
    nc = _get_nc(region_groups, run_blocks)
    res = run_bass_kernel_spmd(
        nc, in_maps, core_ids=list(range(N_CORES)), trace=trace, **kw
    )
    out = np.empty(N_EDGES, np.float32)
    for c in range(N_CORES):
        o = np.asarray(res.results[c]["out"])  # [128, T/128]
        slots = o.T.reshape(-1)  # slot s = col*128 + p
        perm = perms[c]
        mask = perm >= 0
        out[perm[mask]] = slots[mask]
    return out, res


def kernel(x, indices, W1, b1, W2, b2):
    out, _ = run_hw(x, indices, W1, b1, W2, b2, trace=False)
    return out.astype(np.float32)
